# revision 5
# baseline (speedup 1.0000x reference)
"""BlockAttentionResidual routing kernel for 8 Trainium2 NeuronCores.

Computation (per token t): S=9 sources (embedding + 8 block summaries),
RMS-normalized routing keys, dot with a static query -> softmax weights ->
weighted combine of raw sources + routing entropy.

Sharding: data-parallel over the flattened (B*T)=8192 token axis, 1024
tokens per core. Params (query*key_weight, identity) replicated.

Per 128-token tile, per source:
  - ScalarE : Square activation with fused accum_out  -> sum(src^2)   (rms)
  - VectorE : tensor_tensor_reduce(src * qw, sum)     -> dot(src, qw) (logits)
  - TensorE : combine as 9 accumulating float32r matmuls with diagonal
              lhsT = diag(w_s) into PSUM (per-token scaling + sum over s)
Softmax / entropy are tiny [128, 9] ops; 1/sqrt(x) is computed as
exp(-0.5*ln(x)) so ScalarE stays in the natural_log_exp table set.
"""

import numpy as np

import concourse.bass as bass
import concourse.tile as tile
from concourse import bacc, mybir
from concourse import bass_utils

B, T, D, K = 4, 2048, 2048, 8
S = K + 1
N_CORES = 8
BT = B * T
TOK = BT // N_CORES          # tokens per core
P = 128                      # SBUF partitions / tokens per tile
NT = TOK // P                # token tiles per core
BANK = 512                   # fp32 elements per PSUM bank
NB = D // BANK
EPS = 1e-8

F32 = mybir.dt.float32
F32R = mybir.dt.float32r
ALU = mybir.AluOpType
ACT = mybir.ActivationFunctionType
AX = mybir.AxisListType

_nc_cache = []


def _build():
    nc = bacc.Bacc("TRN2", target_bir_lowering=False, debug=False,
                   num_devices=N_CORES)
    emb = nc.dram_tensor("emb", [TOK, D], F32R, kind="ExternalInput").ap()
    blk = nc.dram_tensor("blk", [K, TOK, D], F32R, kind="ExternalInput").ap()
    qw = nc.dram_tensor("qw", [P, D], F32, kind="ExternalInput").ap()
    ident = nc.dram_tensor("ident", [P, P], F32, kind="ExternalInput").ap()
    routed = nc.dram_tensor("routed", [TOK, D], F32, kind="ExternalOutput").ap()
    wout = nc.dram_tensor("wout", [TOK, S], F32, kind="ExternalOutput").ap()
    ent = nc.dram_tensor("ent", [TOK, 1], F32, kind="ExternalOutput").ap()

    with tile.TileContext(nc) as tc:
        with (
            tc.tile_pool(name="const", bufs=1) as const_pool,
            tc.tile_pool(name="src", bufs=2) as src_pool,
            tc.tile_pool(name="scr", bufs=1) as scr_pool,
            tc.tile_pool(name="stat", bufs=2) as stat_pool,
            tc.tile_pool(name="diag", bufs=3) as diag_pool,
            tc.tile_pool(name="outp", bufs=2) as out_pool,
            tc.tile_pool(name="psum", bufs=2, space="PSUM") as psum_pool,
        ):
            qw_t = const_pool.tile([P, D], F32, tag="qw")
            nc.sync.dma_start(qw_t[:], qw[:, :])
            id_t = const_pool.tile([P, P], F32, tag="id")
            nc.sync.dma_start(id_t[:], ident[:, :])
            eps_t = const_pool.tile([P, 1], F32, tag="eps")
            nc.vector.memset(eps_t[:], EPS)

            for t in range(NT):
                r0 = t * P
                # ---- load the 9 sources side by side in the free dim ----
                src = src_pool.tile([P, S * D], F32R, tag="src")
                nc.sync.dma_start(src[:, 0:D], emb[r0:r0 + P, :])
                for k in range(K):
                    nc.sync.dma_start(src[:, (k + 1) * D:(k + 2) * D],
                                      blk[k, r0:r0 + P, :])

                # ---- per-source reductions over D ----
                ss = stat_pool.tile([P, S], F32, tag="ss")    # sum(src^2)
                uu = stat_pool.tile([P, S], F32, tag="uu")    # dot(src, qw)
                sq_scr = scr_pool.tile([P, D], F32, tag="sq")
                tt_scr = scr_pool.tile([P, D], F32, tag="tt")
                for s in range(S):
                    sl = src[:, s * D:(s + 1) * D].bitcast(F32)
                    nc.scalar.activation(sq_scr[:], sl, ACT.Square,
                                         accum_out=ss[:, s:s + 1])
                    nc.vector.scalar_tensor_tensor(
                        out=tt_scr[:], in0=sl, scalar=1.0, in1=qw_t[:],
                        op0=ALU.mult, op1=ALU.mult,
                        accum_out=uu[:, s:s + 1])

                # ---- logits = uu * rsqrt(ss/D + eps) ----
                lnm = stat_pool.tile([P, S], F32, tag="lnm")
                nc.scalar.activation(lnm[:], ss[:], ACT.Ln,
                                     scale=1.0 / D, bias=eps_t[:])
                inv = stat_pool.tile([P, S], F32, tag="inv")
                nc.scalar.activation(inv[:], lnm[:], ACT.Exp, scale=-0.5)
                lg = stat_pool.tile([P, S], F32, tag="lg")
                nc.vector.tensor_tensor(lg[:], uu[:], inv[:], ALU.mult)

                # ---- softmax over the 9 sources ----
                mx = stat_pool.tile([P, 1], F32, tag="mx")
                nc.vector.tensor_reduce(mx[:], lg[:], AX.X, ALU.max)
                nmx = stat_pool.tile([P, 1], F32, tag="nmx")
                nc.vector.tensor_scalar(nmx[:], mx[:], -1.0, None, ALU.mult)
                e9 = stat_pool.tile([P, S], F32, tag="e9")
                s1 = stat_pool.tile([P, 1], F32, tag="s1")
                nc.scalar.activation(e9[:], lg[:], ACT.Exp, bias=nmx[:],
                                     scale=1.0, accum_out=s1[:])
                rs = stat_pool.tile([P, 1], F32, tag="rs")
                nc.vector.reciprocal(rs[:], s1[:])
                w9 = stat_pool.tile([P, S], F32, tag="w9")
                nc.vector.tensor_scalar(w9[:], e9[:], rs[:], None, ALU.mult)
                nc.sync.dma_start(wout[r0:r0 + P, :], w9[:])

                # ---- entropy = ln(S) - sum(e*(lg-mx))/S ----
                x9 = stat_pool.tile([P, S], F32, tag="x9")
                nc.vector.tensor_scalar(x9[:], lg[:], nmx[:], None, ALU.add)
                ex_scr = stat_pool.tile([P, S], F32, tag="ex_scr")
                ex = stat_pool.tile([P, 1], F32, tag="ex")
                nc.vector.scalar_tensor_tensor(
                    out=ex_scr[:], in0=e9[:], scalar=1.0, in1=x9[:],
                    op0=ALU.mult, op1=ALU.mult, accum_out=ex[:])
                lns = stat_pool.tile([P, 1], F32, tag="lns")
                nc.scalar.activation(lns[:], s1[:], ACT.Ln)
                t1 = stat_pool.tile([P, 1], F32, tag="t1")
                nc.vector.tensor_tensor(t1[:], ex[:], rs[:], ALU.mult)
                et = stat_pool.tile([P, 1], F32, tag="et")
                nc.vector.tensor_tensor(et[:], lns[:], t1[:], ALU.subtract)
                nc.sync.dma_start(ent[r0:r0 + P, :], et[:])

                # ---- combine: routed = sum_s w_s * src_s via diag matmuls ----
                ps = psum_pool.tile([P, D], F32, tag="ps")
                for s in range(S):
                    dg = diag_pool.tile([P, P], F32R, tag="dg")
                    nc.vector.tensor_scalar(dg[:], id_t[:], w9[:, s:s + 1],
                                            None, ALU.mult)
                    for b in range(NB):
                        nc.tensor.matmul(
                            ps[:, b * BANK:(b + 1) * BANK],
                            lhsT=dg[:],
                            rhs=src[:, s * D + b * BANK:
                                    s * D + (b + 1) * BANK],
                            start=(s == 0), stop=(s == S - 1))
                ot = out_pool.tile([P, D], F32, tag="ot")
                nc.scalar.copy(ot[:], ps[:])
                nc.sync.dma_start(routed[r0:r0 + P, :], ot[:])

    nc.compile()
    return nc


def _get_nc():
    if not _nc_cache:
        _nc_cache.append(_build())
    return _nc_cache[0]


def _run(embedding, blocks, query, key_weight, **spmd_kwargs):
    emb2 = np.ascontiguousarray(
        np.asarray(embedding, dtype=np.float32).reshape(BT, D))
    blk2 = np.asarray(blocks, dtype=np.float32).reshape(K, BT, D)
    qwv = (np.asarray(query, dtype=np.float32)
           * np.asarray(key_weight, dtype=np.float32))
    qw_b = np.ascontiguousarray(np.broadcast_to(qwv, (P, D)))
    idm = np.eye(P, dtype=np.float32)

    in_maps = []
    for c in range(N_CORES):
        sl = slice(c * TOK, (c + 1) * TOK)
        in_maps.append({
            "emb": emb2[sl],
            "blk": np.ascontiguousarray(blk2[:, sl, :]),
            "qw": qw_b,
            "ident": idm,
        })

    nc = _get_nc()
    res = bass_utils.run_bass_kernel_spmd(nc, in_maps,
                                          core_ids=list(range(N_CORES)),
                                          **spmd_kwargs)
    routed = np.concatenate(
        [res.results[c]["routed"] for c in range(N_CORES)], axis=0
    ).reshape(B, T, D)
    weights = np.concatenate(
        [res.results[c]["wout"] for c in range(N_CORES)], axis=0
    ).reshape(B, T, S)
    entropy = np.concatenate(
        [res.results[c]["ent"] for c in range(N_CORES)], axis=0
    ).reshape(B, T)
    return (routed, weights, entropy), res


def kernel(embedding, blocks, query, key_weight):
    outs, _ = _run(embedding, blocks, query, key_weight)
    return outs
